# revision 1
# baseline (speedup 1.0000x reference)
"""Gemma3 sliding-window attention, tensor-parallel over 8 NeuronCores.

Sharding (per hint): 8 q-heads / 4 kv-heads -> 1 q-head per core,
kv-heads replicated 2x so each core holds its own kv head copy
(column-shard wq/wk/wv), row-shard wo, psum all-reduce after o_proj.
"""
import os
import numpy as np
import jax
import jax.numpy as jnp
from functools import partial
from jax.sharding import Mesh, PartitionSpec as P, NamedSharding

B, S, HID = 2, 2048, 2560
NH, NKV, HD = 8, 4, 256
GROUPS = NH // NKV
SCALE = 256.0 ** -0.5
EPS = 1e-6
WIN = 1024
NCORES = 8

try:
    from jax import shard_map as _shard_map_mod
    shard_map = _shard_map_mod.shard_map if hasattr(_shard_map_mod, "shard_map") else _shard_map_mod
except Exception:
    from jax.experimental.shard_map import shard_map


def _rmsnorm(x, w):
    n = x * jax.lax.rsqrt(jnp.mean(x * x, axis=-1, keepdims=True) + EPS)
    return n * (1.0 + w)


def _rotate_half(x):
    h = x.shape[-1] // 2
    return jnp.concatenate([-x[..., h:], x[..., :h]], axis=-1)


def _core_fn(x, cos, sin, wq, wk, wv, wo, qn, kn):
    # local shards: wq/wk/wv [HID, 1, HD]; wo [1, HD, HID]; x/cos/sin replicated
    wq = wq[:, 0, :]
    wk = wk[:, 0, :]
    wv = wv[:, 0, :]
    wo = wo[0]
    q = jnp.einsum("bsd,dh->bsh", x, wq)          # [B,S,HD]
    k = jnp.einsum("bsd,dh->bsh", x, wk)
    v = jnp.einsum("bsd,dh->bsh", x, wv)
    q = _rmsnorm(q, qn)
    k = _rmsnorm(k, kn)
    q = q * cos + _rotate_half(q) * sin
    k = k * cos + _rotate_half(k) * sin
    # blocked attention over query tiles; sliding window keeps each tile's
    # key span <= QT + WIN
    QT = 512
    nmin = jnp.finfo(jnp.float32).min
    outs = []
    for q0 in range(0, S, QT):
        k0 = max(0, q0 - WIN + 1)
        k0 = (k0 // QT) * QT                      # align tile start
        qa = q[:, q0:q0 + QT]                     # [B,QT,HD]
        ka = k[:, k0:q0 + QT]                     # [B,KT,HD]
        va = v[:, k0:q0 + QT]
        s = jnp.einsum("bqd,bkd->bqk", qa, ka) * SCALE
        i = (q0 + jnp.arange(QT))[:, None]
        j = (k0 + jnp.arange(qa.shape[1] + q0 - k0))[None, :]
        allowed = (j <= i) & ((i - j) < WIN)
        s = jnp.where(allowed, s, nmin)
        p = jax.nn.softmax(s, axis=-1)
        outs.append(jnp.einsum("bqk,bkd->bqd", p, va))
    o = jnp.concatenate(outs, axis=1)             # [B,S,HD]
    part = jnp.einsum("bsd,de->bse", o, wo)       # [B,S,HID]
    return jax.lax.psum(part, "x")


_mesh = None
_jitted = None


def _build():
    global _mesh, _jitted
    devs = jax.devices()[:NCORES]
    _mesh = Mesh(np.array(devs), ("x",))
    fn = shard_map(
        _core_fn,
        mesh=_mesh,
        in_specs=(P(), P(), P(), P(None, "x", None), P(None, "x", None),
                  P(None, "x", None), P("x", None, None), P(), P()),
        out_specs=P(),
    )
    _jitted = jax.jit(fn)
    return _mesh, _jitted


def _prep(hidden_states, cos, sin, wq, wk, wv, wo, q_norm_w, k_norm_w):
    # replicate each kv head GROUPS times so core c owns q-head c and kv-head c//2
    wk3 = wk.reshape(HID, NKV, HD)
    wv3 = wv.reshape(HID, NKV, HD)
    wk8 = np.repeat(wk3, GROUPS, axis=1)          # [HID, 8, HD]
    wv8 = np.repeat(wv3, GROUPS, axis=1)
    wq8 = wq.reshape(HID, NH, HD)
    wo8 = wo.reshape(NH, HD, HID)
    return (np.ascontiguousarray(hidden_states, np.float32),
            np.ascontiguousarray(cos, np.float32),
            np.ascontiguousarray(sin, np.float32),
            np.ascontiguousarray(wq8, np.float32),
            np.ascontiguousarray(wk8, np.float32),
            np.ascontiguousarray(wv8, np.float32),
            np.ascontiguousarray(wo8, np.float32),
            np.ascontiguousarray(q_norm_w, np.float32),
            np.ascontiguousarray(k_norm_w, np.float32))


def kernel(hidden_states, cos, sin, wq, wk, wv, wo, q_norm_w, k_norm_w):
    mesh, jitted = (_mesh, _jitted) if _jitted is not None else _build()
    args = _prep(hidden_states, cos, sin, wq, wk, wv, wo, q_norm_w, k_norm_w)
    specs = (P(), P(), P(), P(None, "x", None), P(None, "x", None),
             P(None, "x", None), P("x", None, None), P(), P())
    put = [jax.device_put(a, NamedSharding(mesh, sp)) for a, sp in zip(args, specs)]
    out = jitted(*put)
    return np.asarray(out)

